# revision 4
# baseline (speedup 1.0000x reference)
"""ConvLSTM cell kernel for Trainium2 (8 NeuronCores), v2.

Sharding: data-parallel over batch B=4 x spatial split of H=64 into 2 halves
(8 shards). The recurrence prevents sharding T. Each core computes its half
with a shrinking row margin (47-t rows at step t) so no cross-core
communication is needed. Bottom halves are row-flipped on the host so a
single SPMD program serves all 8 cores.

v2 design:
- conv h2h runs in fp8e4 with MatmulPerfMode.DoubleRow: the h state lives in
  SBUF as [128, 2, 56, 66] fp8 with partition halves [hpad; hpad+1row] in
  plane j=0 and [hpad+2rows; zero] in plane j=1. One DR matmul per dx tap
  column covers taps (dy0,dx),(dy1,dx),(dy2,dx) at K-effective 256, so the
  whole 3x3 conv is 3 DR matmuls per 128-wide oc tile (+1 bf16 identity
  matmul that adds x into PSUM).
- gates: tile0 = [f; i] via one ACT Sigmoid call straight from PSUM;
  tile1 = [s_o; g] via one ACT Tanh call with per-partition scale
  [0.5 (o) | 1.0 (g)]; o is reconstructed on DVE with a 4x tensor_scalar.
- state chain on DVE with 2x tensor_tensor ops: a=f*c, b=i*g, c=a+b,
  tc=tanh(c) (ACT), h=o*tc.
- h fp8 planes: one gpsimd bf16->fp8 convert into hpad, then two 4x
  uint16-bitcast row-shift copies on DVE for the +1row/+2row planes.
- Work is split into chunks (rows 0..31, 32..R-1) so PSUM per (tau,chunk)
  is <=4 banks and the recurrence tail pipelines across steps.
"""

import sys

sys.path.insert(0, "/opt/trn_rl_repo")

import numpy as np
from ml_dtypes import bfloat16, float8_e4m3fn

HIDDEN = 64
T_STEPS = 16
B = 4
H = 64
W = 64
ROWS = 48        # per-core x rows (32 owned + 16 margin)
OWN = 32
WP = W + 2       # padded row width 66
PR = 56          # fp8 plane rows (pad so plane stride 56*66=3696 is 16-aligned)
XCOLS = ROWS * W

_CACHE = {}


def _build_nc():
    from concourse import bacc, mybir
    from concourse.tile import TileContext

    dt = mybir.dt
    Alu = mybir.AluOpType
    Act = mybir.ActivationFunctionType
    DR = mybir.MatmulPerfMode.DoubleRow

    nc = bacc.Bacc(None, target_bir_lowering=False)

    x_in = nc.dram_tensor("x", [T_STEPS, 2, 128, XCOLS], dt.bfloat16,
                          kind="ExternalInput")
    w_in = nc.dram_tensor("wdr", [128, 6, 2, 128], dt.float8e4,
                          kind="ExternalInput")
    id_in = nc.dram_tensor("ident", [128, 128], dt.bfloat16,
                           kind="ExternalInput")
    sc_in = nc.dram_tensor("scog", [128, 1], dt.float32,
                           kind="ExternalInput")
    hout = nc.dram_tensor("hout", [T_STEPS, 64, OWN * W], dt.bfloat16,
                          kind="ExternalOutput")

    with TileContext(nc) as tc:
        with (
            tc.tile_pool(name="const", bufs=1) as cpool,
            tc.tile_pool(name="state", bufs=1) as spool,
            tc.tile_pool(name="xload", bufs=3) as xpool,
            tc.tile_pool(name="gate", bufs=2) as gpool,
            tc.tile_pool(name="work", bufs=2) as wpool,
            tc.tile_pool(name="ps", bufs=1, space="PSUM") as psp,
        ):
            w_sb = cpool.tile([128, 6, 2, 128], dt.float8e4, tag="w")
            id_sb = cpool.tile([128, 128], dt.bfloat16, tag="id")
            sc_sb = cpool.tile([128, 1], dt.float32, tag="sc")
            nc.sync.dma_start(out=w_sb[:], in_=w_in[:])
            nc.sync.dma_start(out=id_sb[:], in_=id_in[:])
            nc.sync.dma_start(out=sc_sb[:], in_=sc_in[:])

            hb = [
                spool.tile([128, 2, PR, WP], dt.float8e4, tag="hb0", name="hb0"),
                spool.tile([128, 2, PR, WP], dt.float8e4, tag="hb1", name="hb1"),
            ]
            cst = spool.tile([64, XCOLS], dt.bfloat16, tag="cst")
            nc.vector.memset(hb[0][:], 0.0)
            nc.vector.memset(hb[1][:], 0.0)
            nc.vector.memset(cst[:], 0.0)

            ps = [psp.tile([128, 2048], dt.float32, tag=f"ps{i}", name=f"ps{i}")
                  for i in range(2)]

            for t in range(T_STEPS):
                R = 47 - t
                hbr = hb[t % 2]
                hbw = hb[(t + 1) % 2]
                last = t == T_STEPS - 1

                xt = []
                for tau in range(2):
                    xti = xpool.tile([128, XCOLS], dt.bfloat16,
                                     tag=f"x{tau}", name=f"x{tau}")
                    nc.sync.dma_start(out=xti[:, : R * W],
                                      in_=x_in[t, tau][:, : R * W])
                    xt.append(xti)

                fi = gpool.tile([128, XCOLS], dt.bfloat16, tag="fi", name="fi")
                og = gpool.tile([128, XCOLS], dt.bfloat16, tag="og", name="og")
                hcomp = wpool.tile([64, XCOLS], dt.bfloat16, tag="hc",
                                   name="hc")

                # chunks: rows [0, 32) and [32, R)
                chunks = [(0, OWN)] + ([(OWN, R)] if R > OWN else [])
                for ci, (r0, r1) in enumerate(chunks):
                    if last and ci == 1:
                        break  # margin rows unneeded at the final step
                    rows = r1 - r0
                    ncols = rows * W
                    for tau in range(2):
                        P = ps[tau]
                        subs = []
                        q = 0
                        while q < rows:
                            sr = min(8, rows - q)
                            subs.append((q, sr))
                            q += sr
                        # identity matmul adds x (also opens the psum group)
                        for (q, sr) in subs:
                            nc.tensor.matmul(
                                P[:, q * W: (q + sr) * W],
                                lhsT=id_sb[:],
                                rhs=xt[tau][:, (r0 + q) * W: (r0 + q + sr) * W],
                                start=True, stop=(t == 0))
                        if t > 0:
                            for d in range(3):
                                wap = w_sb[:, tau * 3 + d, :, :]
                                for (q, sr) in subs:
                                    nc.tensor.matmul(
                                        P[:, q * W: (q + sr) * W],
                                        lhsT=wap,
                                        rhs=hbr[:, :, r0 + q: r0 + q + sr,
                                                d: d + W],
                                        start=False, stop=(d == 2),
                                        perf_mode=DR)
                        seg = slice(r0 * W, r1 * W)
                        if tau == 0:
                            nc.scalar.activation(fi[:, seg], P[:, :ncols],
                                                 Act.Sigmoid)
                        else:
                            nc.scalar.activation(og[:, seg], P[:, :ncols],
                                                 Act.Tanh, scale=sc_sb[:])

                    # state update chain for this chunk
                    ot = wpool.tile([64, XCOLS], dt.bfloat16, tag="ot",
                                    name="ot")
                    at = wpool.tile([64, XCOLS], dt.bfloat16, tag="at",
                                    name="at")
                    bt = wpool.tile([64, XCOLS], dt.bfloat16, tag="bt",
                                    name="bt")
                    tct = wpool.tile([64, XCOLS], dt.bfloat16, tag="tct",
                                     name="tct")
                    seg = slice(r0 * W, r1 * W)
                    nc.vector.tensor_scalar(ot[:, seg], og[0:64, seg],
                                            0.5, 0.5, Alu.mult, Alu.add)
                    nc.vector.tensor_tensor(at[:, seg], fi[0:64, seg],
                                            cst[:, seg], Alu.mult)
                    nc.vector.tensor_tensor(bt[:, seg], fi[64:128, seg],
                                            og[64:128, seg], Alu.mult)
                    nc.vector.tensor_tensor(cst[:, seg], at[:, seg],
                                            bt[:, seg], Alu.add)
                    nc.scalar.activation(tct[:, seg], cst[:, seg], Act.Tanh)
                    nc.vector.tensor_tensor(hcomp[:, seg], ot[:, seg],
                                            tct[:, seg], Alu.mult)

                    if ci == 0:
                        nc.scalar.dma_start(out=hout[t],
                                            in_=hcomp[:, : OWN * W])

                    if not last:
                        # fp8 shadow planes for the next step's conv
                        src = hcomp[:, seg].rearrange(
                            "p (r c) -> p r c", r=rows)
                        nc.gpsimd.tensor_copy(
                            hbw[0:64, 0, 1 + r0: 1 + r1, 1: 1 + W], src)
                        # +1row plane: dest rows r0..r1-1 <- hpad rows +1
                        nc.vector.tensor_copy(
                            hbw[64:128, 0, r0: r1, :].bitcast(dt.uint16),
                            hbw[0:64, 0, 1 + r0: 1 + r1, :].bitcast(dt.uint16))
                        # +2row plane (j=1, lower half): row k = hpad[k+2]
                        k0 = max(r0 - 1, 0)
                        k1 = r1 - 1
                        nc.vector.tensor_copy(
                            hbw[0:64, 1, k0: k1, :].bitcast(dt.uint16),
                            hbw[0:64, 0, k0 + 2: k1 + 2, :].bitcast(dt.uint16))

    nc.finalize()
    return nc


def _prep_inputs(x, w_h2h):
    """Per-core input maps. Cores: core = b*2 + half."""
    # tau0 tile = [f; i], tau1 tile = [o; g]
    perm = np.concatenate([np.arange(64, 128), np.arange(0, 64),
                           np.arange(128, 192), np.arange(192, 256)])
    w_perm = w_h2h.astype(np.float32)[perm]  # [256, 64, 3, 3]

    def pack_w(wp):
        # wp: [256 (tau-major), 64, 3, 3] -> [128, 6, 2, 128] fp8
        out = np.zeros((128, 6, 2, 128), np.float32)
        for tau in range(2):
            blk = wp[tau * 128: (tau + 1) * 128]  # [128 oc, 64 ic, 3, 3]
            for d in range(3):
                s = tau * 3 + d
                out[0:64, s, 0, :] = blk[:, :, 0, d].T
                out[64:128, s, 0, :] = blk[:, :, 1, d].T
                out[0:64, s, 1, :] = blk[:, :, 2, d].T
        return np.clip(out, -240, 240).astype(float8_e4m3fn)

    w_top = pack_w(w_perm)
    w_bot = pack_w(w_perm[:, :, ::-1, :])
    ident = np.eye(128, dtype=np.float32).astype(bfloat16)
    scog = np.concatenate([np.full((64, 1), 0.5, np.float32),
                           np.ones((64, 1), np.float32)])

    xp = x[:, :, perm]  # [T, B, 256, H, W]
    in_maps = []
    for b in range(B):
        for half in range(2):
            if half == 0:
                xs = xp[:, b, :, 0:ROWS, :]
            else:
                xs = xp[:, b, :, H - ROWS:, :][:, :, ::-1, :]
            xs = np.ascontiguousarray(xs).astype(bfloat16)
            xs = xs.reshape(T_STEPS, 2, 128, XCOLS)
            in_maps.append({
                "x": xs,
                "wdr": w_top if half == 0 else w_bot,
                "ident": ident,
                "scog": scog,
            })
    return in_maps


def kernel(x, w_h2h):
    from concourse import bass_utils

    if "nc" not in _CACHE:
        _CACHE["nc"] = _build_nc()
    nc = _CACHE["nc"]

    in_maps = _prep_inputs(np.asarray(x), np.asarray(w_h2h))
    res = bass_utils.run_bass_kernel_spmd(nc, in_maps,
                                          core_ids=list(range(8)),
                                          **_CACHE.get("run_kwargs", {}))
    _CACHE["last_results"] = res

    out = np.zeros((T_STEPS, B, HIDDEN, H, W), np.float32)
    for b in range(B):
        for half in range(2):
            core = b * 2 + half
            hs = res.results[core]["hout"].astype(np.float32)
            hs = hs.reshape(T_STEPS, HIDDEN, OWN, W)
            if half == 0:
                out[:, b, :, 0:OWN, :] = hs
            else:
                out[:, b, :, OWN:, :] = hs[:, :, ::-1, :]
    return out
